# revision 21
# baseline (speedup 1.0000x reference)
"""GAT multi-head attention layer (nn_GATMutiHeadAttLayer) on 8 Trainium2 cores.

Head-sharded: core h computes head h entirely (no collectives).

Math (per head):
  h = X @ W                       [S, FOUT]
  f1 = h @ a1, f2 = h @ a2        [S]
  e[i,j] = lrelu(f1[i] + f2[j], 0.2), masked by adj[i,j]
  attn = softmax(e, axis=i)  (denominator s[j] = sum_i)
  out = attn @ h, concat heads, ELU.

Device formulation (transposed, j on partitions; fp16 tiles):
  exp(lrelu(z)) = max(exp(z), exp(0.2 z)),  z = f1[i] + f2[j]
  u'[j,i] = max(R1b[i] * eu[j], ev[j])          (TS: mult, max — 2x/4x fast path)
     R1b = exp(0.8 f1) bcast, eu = exp(f2), ev = exp(0.2 f2)
  pb[j,i] = u' * E1sb[i]                        (TT: mult — 2x fast path)
     E1sb = exp(0.2 f1) bcast;  pb = exp(lrelu(z)) unmasked, >= 0
  pb += mask'[j,i]  (mask' in {0, -BIG}) via SWDGE accumulate-DMA (plan D)
                    or a DVE TT add against a DMA'd mask tile (plan Z fallback)
  p = relu(pb), s[j] = sum_i p   (one ACT pass: zeroes masked entries AND
                                  row-sums via the fused accumulator)
  hp = h[band] / s  (GPSIMD normalize_recip: fused divide + reciprocal)
  out^T[o,i] = sum_j hp[j,o] * p[j,i]   (PE, PSUM-accumulated over bands)
  final: ELU(out^T) -> DRAM; host transposes/concats heads.

Preamble (PE fp16):
  wa = W @ [a1|a2] via W^T;  f1 row = wa1^T @ X^T;  bcast via ones-matmul;
  R1b/E1sb exp'd straight out of PSUM.
  [h_band | f2_band] = xt_band^T @ [W | wa2]  (one N=65 matmul per band)
  eu = exp(f2), ev = exp(0.2 f2).

Host prep: X^T, W, W^T, [a1|a2] cast fp16; adj^T cast to fp16 (0/1 exact).
All model compute (matmuls, exp, masking, softmax, ELU) runs on device.
"""

import contextlib
import ctypes
import os
import sys
import types
from contextlib import ExitStack

import numpy as np

import concourse.bass as bass
import concourse.tile as tile
from concourse import bacc, mybir
from concourse import bass_utils

AF = mybir.ActivationFunctionType
ALU = mybir.AluOpType
DT = mybir.dt

S = 4096
FIN = 128
FOUT = 64
H = 8
ALPHA = 0.2

LAST_RESULTS = None  # BassKernelResults of the most recent run (for test harness)

# ---------------------------------------------------------------------------
# NTFF profile hook shim: antenv.axon_hooks is absent in this container; the
# trace=True path of run_bass_kernel_spmd imports it. Recreate it via ctypes
# against libaxon_pjrt.so (same as trn_agent_boot does).
_SO_PATH = "/opt/axon/libaxon_pjrt.so"


def _make_ntff_hook():
    try:
        lib = ctypes.CDLL(_SO_PATH)
    except OSError:
        return None
    if not hasattr(lib, "axon_start_nrt_profile"):
        return None
    lib.axon_start_nrt_profile.argtypes = [ctypes.POINTER(ctypes.c_int64), ctypes.c_size_t]
    lib.axon_start_nrt_profile.restype = ctypes.c_int64
    lib.axon_stop_nrt_profile.argtypes = [ctypes.c_char_p]
    lib.axon_stop_nrt_profile.restype = ctypes.c_int64

    @contextlib.contextmanager
    def _hook(output_dir, device_ids):
        import jax

        jax.devices()
        if device_ids:
            ids = (ctypes.c_int64 * len(device_ids))(*device_ids)
            rc = lib.axon_start_nrt_profile(ids, len(device_ids))
        else:
            rc = lib.axon_start_nrt_profile(None, 0)
        if rc != 0:
            raise RuntimeError(f"axon_start_nrt_profile rc={rc}")
        try:
            yield
        finally:
            n = lib.axon_stop_nrt_profile(str(output_dir).encode())
            if n <= 0:
                print(f"ntff profile: rc={n} (no files?) dir={output_dir}", file=sys.stderr)

    return _hook


def _install_ntff_shim():
    if "antenv.axon_hooks" in sys.modules:
        return
    mod = types.ModuleType("antenv.axon_hooks")
    _hook = _make_ntff_hook()
    mod.get_axon_ntff_profile_hook = lambda: _hook
    mod.set_axon_ntff_profile_hook = lambda h: None
    sys.modules["antenv.axon_hooks"] = mod
    try:
        import antenv

        antenv.axon_hooks = mod
    except ImportError:
        pass


_install_ntff_shim()

# ---------------------------------------------------------------------------

PLAN = os.environ.get("KERNEL_PLAN", "Z")
USE_GP = os.environ.get("KERNEL_GP", "0") == "1"
# Width of the per-band mask-add slice offloaded to GPSIMD (0 = all on DVE)
GPW = int(os.environ.get("KERNEL_GPW", "2304"))


def build_nc(s=S, plan=None):
    """Build + compile the per-core Bass program (same program on all cores)."""
    plan = plan or PLAN
    nb = s // 128     # number of j-bands
    nch = s // 512    # number of 512-wide i-chunks

    nc = bacc.Bacc("TRN2", target_bir_lowering=False, debug=False, enable_asserts=False)

    xt = nc.dram_tensor("xt", [FIN, s], DT.float16, kind="ExternalInput").ap()
    w = nc.dram_tensor("w", [FIN, FOUT], DT.float16, kind="ExternalInput").ap()
    wt = nc.dram_tensor("wt", [FOUT, FIN], DT.float16, kind="ExternalInput").ap()
    a12 = nc.dram_tensor("a12", [FOUT, 2], DT.float16, kind="ExternalInput").ap()
    adjt = nc.dram_tensor("adjt", [s, s], DT.float16, kind="ExternalInput").ap()
    out = nc.dram_tensor("out", [FOUT, s], DT.float32, kind="ExternalOutput").ap()

    with tile.TileContext(nc) as tc, ExitStack() as ctx:
        _body(ctx, tc, nc, xt, w, wt, a12, adjt, out, s, nb, nch, plan)

    if os.environ.get("KERNEL_LDW1", "1") == "1":
        # Mark matmuls whose stationary operand AP repeats the immediately
        # preceding matmul's as non-self-loading (PE keeps the loaded array).
        n_marked = 0
        for blk in nc.m.functions[0].blocks:
            prev_w = None
            for inst in blk.instructions:
                if type(inst).__name__ != "InstMatmult":
                    continue
                wkey = repr(inst.ins[1])
                if prev_w == wkey:
                    inst.ldweights = False
                    n_marked += 1
                prev_w = wkey
        print(f"KERNEL_LDW1: marked {n_marked} matmuls non-self-loading")

    nc.compile()
    return nc


def _body(ctx, tc, nc, xt, w, wt, a12, adjt, out, s, nb, nch, plan):
    f32, f16 = DT.float32, DT.float16

    # ---------------- persistent intermediates (live through main loop) ----
    cpool = ctx.enter_context(tc.tile_pool(name="const", bufs=1))
    r1b_sb = cpool.tile([128, s], f16, tag="r1b")      # exp(0.8 f1[i]) bcast
    e1sb_sb = cpool.tile([128, s], f16, tag="e1sb")    # exp(0.2 f1[i]) bcast
    h_sb = cpool.tile([128, nb * FOUT], f32, tag="h")  # h (f32 for normalize_recip)
    eu_sb = cpool.tile([128, nb], f32, tag="eu")       # exp(f2), band b in col b
    ev_sb = cpool.tile([128, nb], f32, tag="ev")       # exp(0.2 f2)

    # ---------------- preamble (scoped pools, freed before main loop) ------
    with tc.tile_pool(name="pre_sb", bufs=1) as tpool:
        xt_sb = tpool.tile([FIN, s], f16, tag="xt")
        nc.sync.dma_start(xt_sb[:], xt[:])
        w65_sb = tpool.tile([FIN, FOUT + 1], f16, tag="w65")
        nc.sync.dma_start(w65_sb[:, 0:FOUT], w[:])
        wt_sb = tpool.tile([FOUT, FIN], f16, tag="wt")
        nc.sync.dma_start(wt_sb[:], wt[:])
        a12_sb = tpool.tile([FOUT, 2], f16, tag="a12")
        nc.sync.dma_start(a12_sb[:], a12[:])
        wa_sb = tpool.tile([FIN, 2], f16, tag="wa")    # [wa1 | wa2]
        f2_sb = tpool.tile([128, nb], f32, tag="f2")   # f2, band b in col b

        # wa = W @ [a1 | a2]  (contract over FOUT)
        with tc.tile_pool(name="pre_wa", bufs=1, space="PSUM") as pwa:
            wa_ps = pwa.tile([FIN, 2], f32, tag="wa")
            nc.tensor.matmul(wa_ps[:], lhsT=wt_sb[:], rhs=a12_sb[:], start=True, stop=True)
            nc.vector.tensor_copy(wa_sb[:], wa_ps[:])
            nc.vector.tensor_copy(w65_sb[:, FOUT : FOUT + 1], wa_ps[:, 1:2])

        # [h_band | f2_band] = xt_band^T @ [W | wa2]  (one matmul per band)
        with tc.tile_pool(name="pre_h", bufs=2, space="PSUM") as phf:
            for b in range(nb):
                ph = phf.tile([128, FOUT + 1], f32, tag="hf")
                nc.tensor.matmul(ph[:], lhsT=xt_sb[:, bass.ts(b, 128)], rhs=w65_sb[:], start=True, stop=True)
                nc.vector.tensor_copy(h_sb[:, bass.ts(b, FOUT)], ph[:, 0:FOUT])
                nc.vector.tensor_copy(f2_sb[:, b : b + 1], ph[:, FOUT : FOUT + 1])

        # exp of f2 cols (small ACT ops) — early so they don't wait on bcast exps
        nc.scalar.activation(eu_sb[:], f2_sb[:], AF.Exp)
        nc.scalar.activation(ev_sb[:], f2_sb[:], AF.Exp, scale=0.2)

        # zb[p, i] = f1[i] for all partitions p, computed directly via a
        # stride-0 broadcast stationary (wa1 replicated across 128 array
        # columns); Exp applied straight out of PSUM in 1024-wide chunks.
        wa1rep = wa_sb[:, 0:1].broadcast_to([FIN, 128])
        with tc.tile_pool(name="pre_bc", bufs=2, space="PSUM") as pbcp:
            for c in range(s // 1024):
                pbc = pbcp.tile([128, 1024], f32, tag="bc")
                for k in range(2):
                    nc.tensor.matmul(pbc[:, bass.ts(k, 512)], lhsT=wa1rep,
                                     rhs=xt_sb[:, c * 1024 + k * 512 : c * 1024 + (k + 1) * 512],
                                     start=True, stop=True)
                nc.scalar.activation(r1b_sb[:, bass.ts(c, 1024)], pbc[:], AF.Exp, scale=0.8)
                nc.scalar.activation(e1sb_sb[:, bass.ts(c, 1024)], pbc[:], AF.Exp, scale=0.2)

    # ---------------- main loop over j-bands ----------------
    upool = ctx.enter_context(tc.tile_pool(name="umax", bufs=2))
    ppool = ctx.enter_context(tc.tile_pool(name="pmat", bufs=3))
    scrpool = ctx.enter_context(tc.tile_pool(name="scr", bufs=2))
    spool = ctx.enter_context(tc.tile_pool(name="svec", bufs=8))
    hppool = ctx.enter_context(tc.tile_pool(name="hp", bufs=3))
    mpool = ctx.enter_context(tc.tile_pool(name="mask", bufs=3)) if plan == "Z" else None
    qpool = ctx.enter_context(tc.tile_pool(name="qtmp", bufs=2)) if plan == "Z" else None

    mainpsum = ctx.enter_context(tc.tile_pool(name="out_psum", bufs=1, space="PSUM"))
    psum_out = mainpsum.tile([FOUT, s], f32, tag="out")

    for b in range(nb):
        # u' = max(R1b * eu[j], ev[j])   (TS, fast path)
        u_t = upool.tile([128, s], f16, tag="u")
        nc.vector.tensor_scalar(
            out=u_t[:], in0=r1b_sb[:], scalar1=eu_sb[:, b : b + 1],
            scalar2=ev_sb[:, b : b + 1], op0=ALU.mult, op1=ALU.max,
        )
        pb_t = ppool.tile([128, s], f16, tag="pb")
        if plan == "D":
            # pb = u' * E1sb (TT, unmasked exp(lrelu(z)) >= 0), then the mask
            # rides the DMA: pb += mask' with mask' in {0, -BIG}.  Masked-out
            # entries go far negative; the ACT Relu pass below zeroes them.
            nc.vector.tensor_tensor(out=pb_t[:], in0=u_t[:], in1=e1sb_sb[:], op=ALU.mult)
            nc.gpsimd.dma_start(pb_t[:], adjt[bass.ts(b, 128), :], accum_op=ALU.add)
        else:
            # plan Z: mask tile via HWDGE, two TTs on DVE (mult combine, add mask).
            # A GPW-wide slice of the mask-add runs on GPSIMD to offload DVE.
            m_t = mpool.tile([128, s], f16, tag="m")
            nc.sync.dma_start(m_t[:], adjt[bass.ts(b, 128), :])
            q_t = qpool.tile([128, s], f16, tag="q")
            nc.vector.tensor_tensor(out=q_t[:], in0=u_t[:], in1=e1sb_sb[:], op=ALU.mult)
            dw = s - GPW
            nc.vector.tensor_tensor(out=pb_t[:, 0:dw], in0=q_t[:, 0:dw], in1=m_t[:, 0:dw], op=ALU.add)
            if GPW:
                nc.gpsimd.tensor_add(pb_t[:, dw:s], q_t[:, dw:s], m_t[:, dw:s])

        # p = relu(pb) — zeroes masked entries; s[j] = sum_i p (fused accum).
        # The relu'd output is the real P consumed by the matmul.
        s_t = spool.tile([128, 1], f32, tag="s")
        p_t = scrpool.tile([128, s], f16, tag="p")
        nc.scalar.activation(p_t[:], pb_t[:], AF.Relu, accum_out=s_t[:])

        # hp = h[band] / s
        hp_t = hppool.tile([128, FOUT], f16, tag="hp")
        if USE_GP:
            # gpsimd fused divide; also writes 1/s into s_t
            nc.gpsimd.normalize_recip(hp_t[:], h_sb[:, bass.ts(b, FOUT)], s_t[:])
        else:
            rs_t = spool.tile([128, 1], f32, tag="rs")
            nc.vector.reciprocal(rs_t[:], s_t[:])
            nc.scalar.activation(hp_t[:], h_sb[:, bass.ts(b, FOUT)], AF.Copy, scale=rs_t[:])

        # out^T[o, i] += sum_j hp[j, o] * p[j, i]
        for c in range(nch):
            nc.tensor.matmul(
                psum_out[:, bass.ts(c, 512)], lhsT=hp_t[:], rhs=p_t[:, bass.ts(c, 512)],
                start=(b == 0), stop=(b == nb - 1),
            )

    # ---------------- ELU + writeout (chunked to bound SBUF) ----------------
    fpool = ctx.enter_context(tc.tile_pool(name="fin", bufs=2))
    ew = min(1024, s)
    for c in range(s // ew):
        sl = bass.ts(c, ew)
        r_t = fpool.tile([FOUT, ew], f32, tag="relu")
        nc.scalar.activation(r_t[:], psum_out[:, sl], AF.Relu)
        mn_t = fpool.tile([FOUT, ew], f32, tag="min")
        nc.vector.tensor_scalar_min(out=mn_t[:], in0=psum_out[:, sl], scalar1=0.0)
        e_t = fpool.tile([FOUT, ew], f32, tag="exp")
        nc.scalar.activation(e_t[:], mn_t[:], AF.Exp)
        f_t = fpool.tile([FOUT, ew], f32, tag="fin")
        # f = (e - 1) + r
        nc.vector.scalar_tensor_tensor(out=f_t[:], in0=e_t[:], scalar=-1.0, in1=r_t[:], op0=ALU.add, op1=ALU.add)
        nc.sync.dma_start(out[:, sl], f_t[:])


_NC_CACHE = {}


def _get_nc(s=S, plan=None):
    key = (s, plan or PLAN)
    if key not in _NC_CACHE:
        _NC_CACHE[key] = build_nc(s, plan)
    return _NC_CACHE[key]


def kernel(input_seq, adj, W, a_1, a_2):
    """Full-input entry point: shards by head across 8 cores, returns [S, H*FOUT]."""
    global LAST_RESULTS
    X = np.asarray(input_seq)[0]          # [S, FIN] f32
    adjm = np.asarray(adj)[0]             # [S, S] int32
    Wn = np.asarray(W)                    # [H, FIN, FOUT]
    a1n = np.asarray(a_1)                 # [H, FOUT, 1]
    a2n = np.asarray(a_2)                 # [H, FOUT, 1]

    s = X.shape[0]
    xt = np.ascontiguousarray(X.T, dtype=np.float16)
    # mask encoded as {0, -BIG}: p = relu(p + mask') zeroes masked-out entries
    adjt = np.where(np.ascontiguousarray(adjm.T) != 0, np.float16(0.0), np.float16(-60000.0))

    nc = _get_nc(s)
    in_maps = [
        {
            "xt": xt,
            "w": np.ascontiguousarray(Wn[h], dtype=np.float16),
            "wt": np.ascontiguousarray(Wn[h].T, dtype=np.float16),
            "a12": np.ascontiguousarray(
                np.concatenate([a1n[h], a2n[h]], axis=1), dtype=np.float16
            ),
            "adjt": adjt,
        }
        for h in range(H)
    ]
    res = bass_utils.run_bass_kernel_spmd(nc, in_maps, core_ids=list(range(H)))
    LAST_RESULTS = res

    outf = np.empty((s, H * FOUT), dtype=np.float32)
    for h in range(H):
        outf[:, h * FOUT : (h + 1) * FOUT] = res.results[h]["out"].T
    return outf


# revision 22
# speedup vs baseline: 1.4152x; 1.4152x over previous
"""GAT multi-head attention layer (nn_GATMutiHeadAttLayer) on 8 Trainium2 cores.

Head-sharded: core h computes head h entirely (no collectives).

Math (per head):
  h = X @ W                       [S, FOUT]
  f1 = h @ a1, f2 = h @ a2        [S]
  e[i,j] = lrelu(f1[i] + f2[j], 0.2), masked by adj[i,j]
  attn = softmax(e, axis=i)  (denominator s[j] = sum_i)
  out = attn @ h, concat heads, ELU.

Device formulation (transposed, j on partitions; fp16 tiles):
  exp(lrelu(z)) = max(exp(z), exp(0.2 z)),  z = f1[i] + f2[j]
  u'[j,i] = max(R1b[i] * eu[j], ev[j])          (TS: mult, max — 2x/4x fast path)
     R1b = exp(0.8 f1) bcast, eu = exp(f2), ev = exp(0.2 f2)
  pb[j,i] = u' * E1sb[i]                        (TT: mult — 2x fast path)
     E1sb = exp(0.2 f1) bcast;  pb = exp(lrelu(z)) unmasked, >= 0
  pb += mask'[j,i]  (mask' in {0, -BIG}) via SWDGE accumulate-DMA (plan D)
                    or a DVE TT add against a DMA'd mask tile (plan Z fallback)
  p = relu(pb), s[j] = sum_i p   (one ACT pass: zeroes masked entries AND
                                  row-sums via the fused accumulator)
  hp = h[band] / s  (GPSIMD normalize_recip: fused divide + reciprocal)
  out^T[o,i] = sum_j hp[j,o] * p[j,i]   (PE, PSUM-accumulated over bands)
  final: ELU(out^T) -> DRAM; host transposes/concats heads.

Preamble (PE fp16):
  wa = W @ [a1|a2] via W^T;  f1 row = wa1^T @ X^T;  bcast via ones-matmul;
  R1b/E1sb exp'd straight out of PSUM.
  [h_band | f2_band] = xt_band^T @ [W | wa2]  (one N=65 matmul per band)
  eu = exp(f2), ev = exp(0.2 f2).

Host prep: X^T, W, W^T, [a1|a2] cast fp16; adj^T cast to fp16 (0/1 exact).
All model compute (matmuls, exp, masking, softmax, ELU) runs on device.
"""

import contextlib
import ctypes
import os
import sys
import types
from contextlib import ExitStack

import numpy as np

import concourse.bass as bass
import concourse.tile as tile
from concourse import bacc, mybir
from concourse import bass_utils

AF = mybir.ActivationFunctionType
ALU = mybir.AluOpType
DT = mybir.dt

S = 4096
FIN = 128
FOUT = 64
H = 8
ALPHA = 0.2

LAST_RESULTS = None  # BassKernelResults of the most recent run (for test harness)

# ---------------------------------------------------------------------------
# NTFF profile hook shim: antenv.axon_hooks is absent in this container; the
# trace=True path of run_bass_kernel_spmd imports it. Recreate it via ctypes
# against libaxon_pjrt.so (same as trn_agent_boot does).
_SO_PATH = "/opt/axon/libaxon_pjrt.so"


def _make_ntff_hook():
    try:
        lib = ctypes.CDLL(_SO_PATH)
    except OSError:
        return None
    if not hasattr(lib, "axon_start_nrt_profile"):
        return None
    lib.axon_start_nrt_profile.argtypes = [ctypes.POINTER(ctypes.c_int64), ctypes.c_size_t]
    lib.axon_start_nrt_profile.restype = ctypes.c_int64
    lib.axon_stop_nrt_profile.argtypes = [ctypes.c_char_p]
    lib.axon_stop_nrt_profile.restype = ctypes.c_int64

    @contextlib.contextmanager
    def _hook(output_dir, device_ids):
        import jax

        jax.devices()
        if device_ids:
            ids = (ctypes.c_int64 * len(device_ids))(*device_ids)
            rc = lib.axon_start_nrt_profile(ids, len(device_ids))
        else:
            rc = lib.axon_start_nrt_profile(None, 0)
        if rc != 0:
            raise RuntimeError(f"axon_start_nrt_profile rc={rc}")
        try:
            yield
        finally:
            n = lib.axon_stop_nrt_profile(str(output_dir).encode())
            if n <= 0:
                print(f"ntff profile: rc={n} (no files?) dir={output_dir}", file=sys.stderr)

    return _hook


def _install_ntff_shim():
    if "antenv.axon_hooks" in sys.modules:
        return
    mod = types.ModuleType("antenv.axon_hooks")
    _hook = _make_ntff_hook()
    mod.get_axon_ntff_profile_hook = lambda: _hook
    mod.set_axon_ntff_profile_hook = lambda h: None
    sys.modules["antenv.axon_hooks"] = mod
    try:
        import antenv

        antenv.axon_hooks = mod
    except ImportError:
        pass


_install_ntff_shim()

# ---------------------------------------------------------------------------

PLAN = os.environ.get("KERNEL_PLAN", "Z")
USE_GP = os.environ.get("KERNEL_GP", "0") == "1"
# Width of the per-band mask-add slice offloaded to GPSIMD (0 = all on DVE)
GPW = int(os.environ.get("KERNEL_GPW", "0"))


def build_nc(s=S, plan=None):
    """Build + compile the per-core Bass program (same program on all cores)."""
    plan = plan or PLAN
    nb = s // 128     # number of j-bands
    nch = s // 512    # number of 512-wide i-chunks

    nc = bacc.Bacc("TRN2", target_bir_lowering=False, debug=False, enable_asserts=False)

    xt = nc.dram_tensor("xt", [FIN, s], DT.float16, kind="ExternalInput").ap()
    w = nc.dram_tensor("w", [FIN, FOUT], DT.float16, kind="ExternalInput").ap()
    wt = nc.dram_tensor("wt", [FOUT, FIN], DT.float16, kind="ExternalInput").ap()
    a12 = nc.dram_tensor("a12", [FOUT, 2], DT.float16, kind="ExternalInput").ap()
    adjt = nc.dram_tensor("adjt", [s, s], DT.float16, kind="ExternalInput").ap()
    out = nc.dram_tensor("out", [FOUT, s], DT.float32, kind="ExternalOutput").ap()

    with tile.TileContext(nc) as tc, ExitStack() as ctx:
        _body(ctx, tc, nc, xt, w, wt, a12, adjt, out, s, nb, nch, plan)

    if os.environ.get("KERNEL_LDW1", "1") == "1":
        # Mark matmuls whose stationary operand AP repeats the immediately
        # preceding matmul's as non-self-loading (PE keeps the loaded array).
        n_marked = 0
        for blk in nc.m.functions[0].blocks:
            prev_w = None
            for inst in blk.instructions:
                if type(inst).__name__ != "InstMatmult":
                    continue
                wkey = repr(inst.ins[1])
                if prev_w == wkey:
                    inst.ldweights = False
                    n_marked += 1
                prev_w = wkey
        print(f"KERNEL_LDW1: marked {n_marked} matmuls non-self-loading")

    nc.compile()
    return nc


def _body(ctx, tc, nc, xt, w, wt, a12, adjt, out, s, nb, nch, plan):
    f32, f16 = DT.float32, DT.float16

    # ---------------- persistent intermediates (live through main loop) ----
    cpool = ctx.enter_context(tc.tile_pool(name="const", bufs=1))
    r1b_sb = cpool.tile([128, s], f16, tag="r1b")      # exp(0.8 f1[i]) bcast
    e1sb_sb = cpool.tile([128, s], f16, tag="e1sb")    # exp(0.2 f1[i]) bcast
    h_sb = cpool.tile([128, nb * FOUT], f32, tag="h")  # h (f32 for normalize_recip)
    eu_sb = cpool.tile([128, nb], f32, tag="eu")       # exp(f2), band b in col b
    ev_sb = cpool.tile([128, nb], f32, tag="ev")       # exp(0.2 f2)

    # ---------------- preamble (scoped pools, freed before main loop) ------
    with tc.tile_pool(name="pre_sb", bufs=1) as tpool:
        xt_sb = tpool.tile([FIN, s], f16, tag="xt")
        nc.sync.dma_start(xt_sb[:], xt[:])
        w65_sb = tpool.tile([FIN, FOUT + 1], f16, tag="w65")
        nc.sync.dma_start(w65_sb[:, 0:FOUT], w[:])
        wt_sb = tpool.tile([FOUT, FIN], f16, tag="wt")
        nc.sync.dma_start(wt_sb[:], wt[:])
        a12_sb = tpool.tile([FOUT, 2], f16, tag="a12")
        nc.sync.dma_start(a12_sb[:], a12[:])
        wa_sb = tpool.tile([FIN, 2], f16, tag="wa")    # [wa1 | wa2]
        f2_sb = tpool.tile([128, nb], f32, tag="f2")   # f2, band b in col b

        # wa = W @ [a1 | a2]  (contract over FOUT)
        with tc.tile_pool(name="pre_wa", bufs=1, space="PSUM") as pwa:
            wa_ps = pwa.tile([FIN, 2], f32, tag="wa")
            nc.tensor.matmul(wa_ps[:], lhsT=wt_sb[:], rhs=a12_sb[:], start=True, stop=True)
            nc.vector.tensor_copy(wa_sb[:], wa_ps[:])
            nc.vector.tensor_copy(w65_sb[:, FOUT : FOUT + 1], wa_ps[:, 1:2])

        # [h_band | f2_band] = xt_band^T @ [W | wa2]  (one matmul per band)
        with tc.tile_pool(name="pre_h", bufs=2, space="PSUM") as phf:
            for b in range(nb):
                ph = phf.tile([128, FOUT + 1], f32, tag="hf")
                nc.tensor.matmul(ph[:], lhsT=xt_sb[:, bass.ts(b, 128)], rhs=w65_sb[:], start=True, stop=True)
                nc.vector.tensor_copy(h_sb[:, bass.ts(b, FOUT)], ph[:, 0:FOUT])
                nc.vector.tensor_copy(f2_sb[:, b : b + 1], ph[:, FOUT : FOUT + 1])

        # exp of f2 cols (small ACT ops) — early so they don't wait on bcast exps
        nc.scalar.activation(eu_sb[:], f2_sb[:], AF.Exp)
        nc.scalar.activation(ev_sb[:], f2_sb[:], AF.Exp, scale=0.2)

        # zb[p, i] = f1[i] for all partitions p, computed directly via a
        # stride-0 broadcast stationary (wa1 replicated across 128 array
        # columns); Exp applied straight out of PSUM in 1024-wide chunks.
        wa1rep = wa_sb[:, 0:1].broadcast_to([FIN, 128])
        with tc.tile_pool(name="pre_bc", bufs=2, space="PSUM") as pbcp:
            for c in range(s // 1024):
                pbc = pbcp.tile([128, 1024], f32, tag="bc")
                for k in range(2):
                    nc.tensor.matmul(pbc[:, bass.ts(k, 512)], lhsT=wa1rep,
                                     rhs=xt_sb[:, c * 1024 + k * 512 : c * 1024 + (k + 1) * 512],
                                     start=True, stop=True)
                nc.scalar.activation(r1b_sb[:, bass.ts(c, 1024)], pbc[:], AF.Exp, scale=0.8)
                nc.scalar.activation(e1sb_sb[:, bass.ts(c, 1024)], pbc[:], AF.Exp, scale=0.2)

    # ---------------- main loop over j-bands ----------------
    upool = ctx.enter_context(tc.tile_pool(name="umax", bufs=2))
    ppool = ctx.enter_context(tc.tile_pool(name="pmat", bufs=3))
    scrpool = ctx.enter_context(tc.tile_pool(name="scr", bufs=2))
    spool = ctx.enter_context(tc.tile_pool(name="svec", bufs=8))
    hppool = ctx.enter_context(tc.tile_pool(name="hp", bufs=3))
    mpool = ctx.enter_context(tc.tile_pool(name="mask", bufs=3)) if plan == "Z" else None
    qpool = ctx.enter_context(tc.tile_pool(name="qtmp", bufs=2)) if plan == "Z" else None

    mainpsum = ctx.enter_context(tc.tile_pool(name="out_psum", bufs=1, space="PSUM"))
    psum_out = mainpsum.tile([FOUT, s], f32, tag="out")

    for b in range(nb):
        # u' = max(R1b * eu[j], ev[j])   (TS, fast path)
        u_t = upool.tile([128, s], f16, tag="u")
        nc.vector.tensor_scalar(
            out=u_t[:], in0=r1b_sb[:], scalar1=eu_sb[:, b : b + 1],
            scalar2=ev_sb[:, b : b + 1], op0=ALU.mult, op1=ALU.max,
        )
        pb_t = ppool.tile([128, s], f16, tag="pb")
        if plan == "D":
            # pb = u' * E1sb (TT, unmasked exp(lrelu(z)) >= 0), then the mask
            # rides the DMA: pb += mask' with mask' in {0, -BIG}.  Masked-out
            # entries go far negative; the ACT Relu pass below zeroes them.
            nc.vector.tensor_tensor(out=pb_t[:], in0=u_t[:], in1=e1sb_sb[:], op=ALU.mult)
            nc.gpsimd.dma_start(pb_t[:], adjt[bass.ts(b, 128), :], accum_op=ALU.add)
        else:
            # plan Z: mask tile via HWDGE, two TTs on DVE (mult combine, add mask).
            # A GPW-wide slice of the mask-add runs on GPSIMD to offload DVE.
            m_t = mpool.tile([128, s], f16, tag="m")
            nc.sync.dma_start(m_t[:], adjt[bass.ts(b, 128), :])
            q_t = qpool.tile([128, s], f16, tag="q")
            nc.vector.tensor_tensor(out=q_t[:], in0=u_t[:], in1=e1sb_sb[:], op=ALU.mult)
            dw = s - GPW
            nc.vector.tensor_tensor(out=pb_t[:, 0:dw], in0=q_t[:, 0:dw], in1=m_t[:, 0:dw], op=ALU.add)
            if GPW:
                nc.gpsimd.tensor_add(pb_t[:, dw:s], q_t[:, dw:s], m_t[:, dw:s])

        # p = relu(pb) — zeroes masked entries; s[j] = sum_i p (fused accum).
        # The relu'd output is the real P consumed by the matmul.
        s_t = spool.tile([128, 1], f32, tag="s")
        p_t = scrpool.tile([128, s], f16, tag="p")
        nc.scalar.activation(p_t[:], pb_t[:], AF.Relu, accum_out=s_t[:])

        # hp = h[band] / s
        hp_t = hppool.tile([128, FOUT], f16, tag="hp")
        if USE_GP:
            # gpsimd fused divide; also writes 1/s into s_t
            nc.gpsimd.normalize_recip(hp_t[:], h_sb[:, bass.ts(b, FOUT)], s_t[:])
        else:
            rs_t = spool.tile([128, 1], f32, tag="rs")
            nc.vector.reciprocal(rs_t[:], s_t[:])
            nc.scalar.activation(hp_t[:], h_sb[:, bass.ts(b, FOUT)], AF.Copy, scale=rs_t[:])

        # out^T[o, i] += sum_j hp[j, o] * p[j, i]
        for c in range(nch):
            nc.tensor.matmul(
                psum_out[:, bass.ts(c, 512)], lhsT=hp_t[:], rhs=p_t[:, bass.ts(c, 512)],
                start=(b == 0), stop=(b == nb - 1),
            )

    # ---------------- ELU + writeout (chunked to bound SBUF) ----------------
    fpool = ctx.enter_context(tc.tile_pool(name="fin", bufs=2))
    ew = min(1024, s)
    for c in range(s // ew):
        sl = bass.ts(c, ew)
        r_t = fpool.tile([FOUT, ew], f32, tag="relu")
        nc.scalar.activation(r_t[:], psum_out[:, sl], AF.Relu)
        mn_t = fpool.tile([FOUT, ew], f32, tag="min")
        nc.vector.tensor_scalar_min(out=mn_t[:], in0=psum_out[:, sl], scalar1=0.0)
        e_t = fpool.tile([FOUT, ew], f32, tag="exp")
        nc.scalar.activation(e_t[:], mn_t[:], AF.Exp)
        f_t = fpool.tile([FOUT, ew], f32, tag="fin")
        # f = (e - 1) + r
        nc.vector.scalar_tensor_tensor(out=f_t[:], in0=e_t[:], scalar=-1.0, in1=r_t[:], op0=ALU.add, op1=ALU.add)
        nc.sync.dma_start(out[:, sl], f_t[:])


_NC_CACHE = {}


def _get_nc(s=S, plan=None):
    key = (s, plan or PLAN)
    if key not in _NC_CACHE:
        _NC_CACHE[key] = build_nc(s, plan)
    return _NC_CACHE[key]


def kernel(input_seq, adj, W, a_1, a_2):
    """Full-input entry point: shards by head across 8 cores, returns [S, H*FOUT]."""
    global LAST_RESULTS
    X = np.asarray(input_seq)[0]          # [S, FIN] f32
    adjm = np.asarray(adj)[0]             # [S, S] int32
    Wn = np.asarray(W)                    # [H, FIN, FOUT]
    a1n = np.asarray(a_1)                 # [H, FOUT, 1]
    a2n = np.asarray(a_2)                 # [H, FOUT, 1]

    s = X.shape[0]
    xt = np.ascontiguousarray(X.T, dtype=np.float16)
    # mask encoded as {0, -BIG}: p = relu(p + mask') zeroes masked-out entries
    adjt = np.where(np.ascontiguousarray(adjm.T) != 0, np.float16(0.0), np.float16(-60000.0))

    nc = _get_nc(s)
    in_maps = [
        {
            "xt": xt,
            "w": np.ascontiguousarray(Wn[h], dtype=np.float16),
            "wt": np.ascontiguousarray(Wn[h].T, dtype=np.float16),
            "a12": np.ascontiguousarray(
                np.concatenate([a1n[h], a2n[h]], axis=1), dtype=np.float16
            ),
            "adjt": adjt,
        }
        for h in range(H)
    ]
    res = bass_utils.run_bass_kernel_spmd(nc, in_maps, core_ids=list(range(H)))
    LAST_RESULTS = res

    outf = np.empty((s, H * FOUT), dtype=np.float32)
    for h in range(H):
        outf[:, h * FOUT : (h + 1) * FOUT] = res.results[h]["out"].T
    return outf
